# revision 18
# baseline (speedup 1.0000x reference)
"""Bahdanau attention kernel for Trainium2 (Bass/Tile), data-parallel over batch.

BASELINE (132.7us) — reconstructed copy for A/B fallback.
"""

import sys

if "/opt/trn_rl_repo" not in sys.path:
    sys.path.insert(0, "/opt/trn_rl_repo")

from contextlib import ExitStack

import numpy as np

import concourse.tile as tile
from concourse import bacc, masks, mybir
from concourse.bass_utils import run_bass_kernel_spmd

N_CORES = 8
B, S, Q = 64, 2048, 512
BL = B // N_CORES          # local batches per core
QC = Q // 128              # 128-wide q chunks
SB = 1024                  # s-block width (tanh tile columns)
NSB = S // SB
ST = SB // 128             # 128-row s sub-tiles per block
MMN = 512                  # matmul moving free dim (one PSUM bank)
NH = SB // MMN             # matmul column halves per block
PT = 4                     # s sub-tiles per pm DMA tile ([128, PT*Q] = 8KB/part)

F32 = mybir.dt.float32
F32R = mybir.dt.float32r
I32 = mybir.dt.int32

MM_MODE = "f32r"
TR_F32R = True
DVE_B = 2
PE_B = BL - DVE_B

_CACHE = {}


def _build():
    nc = bacc.Bacc(
        "TRN2",
        target_bir_lowering=False,
        debug=False,
        enable_asserts=False,
        num_devices=N_CORES,
    )
    pm_dt = F32R if TR_F32R else F32
    pm_d = nc.dram_tensor("pm", [BL, S, Q], pm_dt, kind="ExternalInput").ap()
    q_d = nc.dram_tensor("q", [BL, Q], F32, kind="ExternalInput").ap()
    mask_d = nc.dram_tensor("mask", [BL, S], I32, kind="ExternalInput").ap()
    wq_d = nc.dram_tensor("wq", [Q, Q], F32, kind="ExternalInput").ap()
    we_d = nc.dram_tensor("we", [Q], F32, kind="ExternalInput").ap()
    attn_d = nc.dram_tensor("attn", [BL, S], F32, kind="ExternalOutput").ap()

    mm_dt = {"f32r": F32R, "bf16": mybir.dt.bfloat16, "f32": F32}[MM_MODE]
    tanh = mybir.ActivationFunctionType.Tanh
    exp = mybir.ActivationFunctionType.Exp

    with tile.TileContext(nc) as tc, ExitStack() as ctx:
        const = ctx.enter_context(tc.tile_pool(name="const", bufs=1))
        setup = ctx.enter_context(tc.tile_pool(name="setup", bufs=1))
        pmp = ctx.enter_context(tc.tile_pool(name="pmp", bufs=9))
        thp = ctx.enter_context(tc.tile_pool(name="thp", bufs=4))
        ptp = ctx.enter_context(tc.tile_pool(name="ptp", bufs=3, space="PSUM"))
        ep = ctx.enter_context(tc.tile_pool(name="ep", bufs=1, space="PSUM"))
        outp = ctx.enter_context(tc.tile_pool(name="outp", bufs=1))

        ident = const.tile([128, 128], F32)
        masks.make_identity(nc, ident[:])
        if TR_F32R:
            ident_r = const.tile([128, 128], F32R)
            nc.vector.tensor_copy(ident_r[:], ident[:])
        else:
            ident_r = ident

        # ---- setup: weights, query, mask ---------------------------------
        # host passes Wq PRE-TRANSPOSED (WqT[q, d] = Wq[d, q]) so the
        # q-on-partitions layout loads directly, skipping 16 PE transposes
        # on the critical setup chain that gates the first tanh:
        # wqT[p, c*Q + d] = WqT[c*128 + p, d] = Wq[d, c*128 + p]
        wqT = setup.tile([128, QC * Q], F32)
        nc.sync.dma_start(
            wqT[:].rearrange("p (c q) -> p c q", c=QC),
            wq_d.rearrange("(c p) q -> p c q", p=128),
        )
        q_nat = setup.tile([BL, Q], F32)
        nc.sync.dma_start(q_nat[:], q_d[:])
        weT = setup.tile([128, QC], F32)
        nc.sync.dma_start(weT[:], we_d.rearrange("(c p) -> p c", p=128))
        WP = 2 * PE_B - 1
        we_pad = setup.tile([128, QC * WP], F32)
        nc.vector.memset(we_pad[:], 0.0)
        for qc in range(QC):
            nc.vector.tensor_copy(
                we_pad[:, qc * WP + PE_B - 1 : qc * WP + PE_B], weT[:, qc : qc + 1]
            )
        we_mm = setup.tile([128, QC * WP], mm_dt)
        nc.vector.tensor_copy(we_mm[:], we_pad[:])
        mask_i = setup.tile([BL, S], I32)
        nc.sync.dma_start(mask_i[:], mask_d[:])
        mask_f = setup.tile([BL, S], F32)
        nc.vector.tensor_copy(mask_f[:], mask_i[:])

        # ---- transpose query so q lands on partitions --------------------
        qT = setup.tile([128, QC * BL], F32)
        for qc in range(QC):
            pt = ptp.tile([128, BL], F32, tag="pt", name=f"qt_{qc}")
            nc.tensor.transpose(
                pt[:], q_nat[:, qc * 128 : (qc + 1) * 128], ident[0:BL, 0:BL]
            )
            nc.vector.tensor_copy(qT[:, qc * BL : (qc + 1) * BL], pt[:])

        # ---- pqT ----------------------------------------------------------
        pqT = setup.tile([128, QC * BL], F32)
        for dc in range(QC):
            acc = ep.tile([128, BL], F32, tag="e", name=f"pq_{dc}")
            for qc in range(QC):
                nc.tensor.matmul(
                    acc[:],
                    wqT[:, qc * Q + dc * 128 : qc * Q + (dc + 1) * 128],
                    qT[:, qc * BL : (qc + 1) * BL],
                    start=(qc == 0),
                    stop=(qc == QC - 1),
                )
            nc.vector.tensor_copy(pqT[:, dc * BL : (dc + 1) * BL], acc[:])

        # ---- DVE-path constants ------------------------------------------
        ones_f = setup.tile([1, 128], F32)
        nc.vector.memset(ones_f[:], 1.0)
        ones_r = setup.tile([1, 128], mm_dt)
        nc.vector.tensor_copy(ones_r[:], ones_f[:])
        we_row = setup.tile([1, Q], F32)
        nc.sync.dma_start(we_row[:], we_d.rearrange("(o q) -> o q", o=1))
        we_row_r = setup.tile([1, Q], mm_dt)
        nc.vector.tensor_copy(we_row_r[:], we_row[:])
        bc_ps = ptp.tile([128, Q], F32, tag="pt", name="bc_we")
        nc.tensor.matmul(bc_ps[:], ones_r[:], we_row_r[:], start=True, stop=True)
        we_bc = setup.tile([128, Q], F32)
        nc.vector.tensor_copy(we_bc[:], bc_ps[:])

        pq_bc = {}
        for b in range(PE_B, BL):
            row_ps = ptp.tile([1, Q], F32, tag="pt", name=f"rps_{b}")
            for qc in range(QC):
                nc.tensor.transpose(
                    row_ps[:, qc * 128 : (qc + 1) * 128],
                    pqT[:, qc * BL + b : qc * BL + b + 1],
                    ident[:],
                )
            pq_row = setup.tile([1, Q], mm_dt, name=f"pqrow_{b}")
            nc.vector.tensor_copy(pq_row[:], row_ps[:])
            bc2 = ptp.tile([128, Q], F32, tag="pt", name=f"bc_{b}")
            nc.tensor.matmul(bc2[:], ones_r[:], pq_row[:], start=True, stop=True)
            t_bc = setup.tile([128, Q], F32, name=f"pqbc_{b}")
            nc.vector.tensor_copy(t_bc[:], bc2[:])
            pq_bc[b] = t_bc

        # ---- main loop ----------------------------------------------------
        p_m = outp.tile([BL, S], F32)
        z_part = outp.tile([BL, NSB], F32)

        for sb in range(NSB):
            e_ps = ep.tile([PE_B, SB], F32, tag="e", name=f"e_{sb}")
            p_e = outp.tile([BL, SB], F32, tag="pe", bufs=2, name=f"pe_{sb}")
            pm_ts = {}

            def get_pm(b, half):
                if (b, half) not in pm_ts:
                    pm_t = pmp.tile(
                        [128, PT * Q], pm_dt, tag="pm", name=f"pm_{b}_{sb}_{half}"
                    )
                    s0 = sb * SB + half * PT * 128
                    nc.sync.dma_start(
                        pm_t[:].rearrange("p (t q) -> p t q", t=PT),
                        pm_d[b, s0 : s0 + PT * 128, :].rearrange(
                            "(t p) q -> p t q", p=128
                        ),
                    )
                    pm_ts[(b, half)] = pm_t
                return pm_ts[(b, half)]

            pe_units = [("pe", b, qc) for b in range(PE_B) for qc in range(QC)]
            dve_units = [("dve", b, t) for b in range(PE_B, BL) for t in range(ST)]
            dve_scale = 0.85 if sb == NSB - 1 else 1.0
            keyed = [((i + 0.5) / len(pe_units), u) for i, u in enumerate(pe_units)]
            keyed += [(dve_scale * (i + 0.5) / max(1, len(dve_units)), u)
                      for i, u in enumerate(dve_units)]
            units = [u for _, u in sorted(keyed, key=lambda x: x[0])]

            ecols = {}
            for b in range(PE_B, BL):
                ecols[b] = thp.tile([128, ST], F32, tag=f"ecol{b - PE_B}",
                                    bufs=2, name=f"ec_{b}_{sb}")

            for kind, b, j in units:
                if kind == "pe":
                    qc = j
                    pt = ptp.tile([128, SB], pm_dt, tag="pt", name=f"pt_{b}_{sb}_{qc}")
                    for t in range(ST):
                        pm_t = get_pm(b, t // PT)
                        tl = t % PT
                        nc.tensor.transpose(
                            pt[:, t * 128 : (t + 1) * 128],
                            pm_t[:, tl * Q + qc * 128 : tl * Q + (qc + 1) * 128],
                            ident_r[:],
                        )
                    th = thp.tile([128, SB], mm_dt, tag="th", name=f"th_{b}_{sb}_{qc}")
                    nc.scalar.activation(
                        th[:], pt[:], tanh,
                        bias=pqT[:, qc * BL + b : qc * BL + b + 1], scale=1.0,
                    )
                    for h in range(NH):
                        nc.tensor.matmul(
                            e_ps[:, h * MMN : (h + 1) * MMN],
                            we_mm[:, qc * WP + PE_B - 1 - b : qc * WP + 2 * PE_B - 1 - b],
                            th[:, h * MMN : (h + 1) * MMN],
                            start=(b == 0 and qc == 0),
                            stop=(b == PE_B - 1 and qc == QC - 1),
                        )
                else:
                    t = j
                    pm_t = get_pm(b, t // PT)
                    tl = t % PT
                    ta = thp.tile([128, Q], F32, tag="ta", bufs=3,
                                  name=f"ta_{b}_{sb}_{t}")
                    nc.vector.tensor_add(
                        ta[:], pm_t[:, tl * Q : (tl + 1) * Q].bitcast(F32),
                        pq_bc[b][:],
                    )
                    tt = thp.tile([128, Q], F32, tag="tt", bufs=3,
                                  name=f"tt_{b}_{sb}_{t}")
                    nc.scalar.activation(tt[:], ta[:], tanh)
                    sc = thp.tile([128, Q], F32, tag="sc", bufs=2,
                                  name=f"sc_{b}_{sb}_{t}")
                    nc.vector.tensor_mul(sc[:], tt[:], we_bc[:])
                    nc.vector.tensor_reduce(
                        ecols[b][:, t : t + 1], sc[:],
                        axis=mybir.AxisListType.X, op=mybir.AluOpType.add,
                    )

            for b in range(PE_B, BL):
                ecps = ptp.tile([ST, 128], F32, tag="pt", name=f"ecp_{b}_{sb}")
                nc.tensor.transpose(ecps[:], ecols[b][:], ident[:])
                ex4 = thp.tile([ST, 128], F32, tag="ex4", bufs=2,
                               name=f"ex_{b}_{sb}")
                nc.scalar.activation(ex4[:], ecps[:], exp)
                nc.gpsimd.dma_start(p_e[b : b + 1, :], ex4[:])
            nc.scalar.activation(p_e[0:PE_B, :], e_ps[:], exp)
            nc.vector.tensor_mul(p_m[:, sb * SB : (sb + 1) * SB], p_e[:],
                                 mask_f[:, sb * SB : (sb + 1) * SB])
            nc.vector.tensor_reduce(
                z_part[:, sb : sb + 1], p_m[:, sb * SB : (sb + 1) * SB],
                axis=mybir.AxisListType.X, op=mybir.AluOpType.add,
            )

        # ---- finish softmax ----------------------------------------------
        z = outp.tile([BL, 1], F32)
        nc.vector.tensor_reduce(z[:], z_part[:], axis=mybir.AxisListType.X,
                                op=mybir.AluOpType.add)
        zr = outp.tile([BL, 1], F32)
        nc.vector.reciprocal(zr[:], z[:])
        a_t = outp.tile([BL, S], F32)
        for h in range(2):
            hs = S // 2
            nc.vector.tensor_scalar(
                a_t[:, h * hs : (h + 1) * hs], p_m[:, h * hs : (h + 1) * hs],
                zr[:], None, op0=mybir.AluOpType.mult,
            )
            nc.gpsimd.dma_start(attn_d[:, h * hs : (h + 1) * hs],
                                a_t[:, h * hs : (h + 1) * hs])

    nc.compile()
    return nc


def _get_nc():
    if "nc" not in _CACHE:
        _CACHE["nc"] = _build()
    return _CACHE["nc"]


def _make_in_maps(query, projected_memory, mask, Wq, We):
    query = np.asarray(query, dtype=np.float32)
    pm = np.asarray(projected_memory, dtype=np.float32)
    mask = np.asarray(mask, dtype=np.int32)
    wq = np.ascontiguousarray(np.asarray(Wq, dtype=np.float32))
    we = np.ascontiguousarray(np.asarray(We, dtype=np.float32))
    in_maps = []
    for i in range(N_CORES):
        lo, hi = i * BL, (i + 1) * BL
        in_maps.append(
            {
                "pm": np.ascontiguousarray(pm[lo:hi]),
                "q": np.ascontiguousarray(query[0, lo:hi, :]),
                "mask": np.ascontiguousarray(mask[lo:hi]),
                "wq": np.ascontiguousarray(wq.T),
                "we": we,
            }
        )
    return in_maps


def run_spmd(query, projected_memory, mask, Wq, We, **spmd_kwargs):
    nc = _get_nc()
    in_maps = _make_in_maps(query, projected_memory, mask, Wq, We)
    return run_bass_kernel_spmd(nc, in_maps, list(range(N_CORES)), **spmd_kwargs)


def kernel(query, projected_memory, mask, Wq, We):
    res = run_spmd(query, projected_memory, mask, Wq, We)
    attn = np.concatenate([res.results[i]["attn"] for i in range(N_CORES)], axis=0)
    return attn[:, None, :].astype(np.float32)
